# revision 1
# baseline (speedup 1.0000x reference)
"""MoE block (top-2 routed 3x3 conv experts) Trainium2 Bass kernel.

Strategy: data-parallel over batch, 2 samples per core on 8 cores.
Since the conv is linear in the kernel, combine the top-2 expert kernels
with the routing probabilities first (w_comb = sum_e p_e * W_e), then do a
single 3x3 SAME conv per sample, plus bias + residual.

Conv-as-matmul layout: x is stored zero-padded [130x130] per channel in
SBUF, flat, with partitions 0-63 = channels and partitions 64-127 = the
same channels shifted by +2 pixels. A single [128, 4x128] rhs read then
provides taps (dy,-1) on top and (dy,+1) on the bottom half, so the six
dx=+-1 taps are three K=128 matmuls; the three dx=0 taps are K=128
matmuls with zero weights on the bottom half (K=64 matmuls measure ~1.8x
slower). All conv matmuls run in float32r (single-pass fp32 mode,
~235ns per [128,64]x[128,512] MM).

DMA: each engine's DGE lane moves ~150-190 GB/s and its transfers
serialize, so: SP lane = top halves + expert weights A + even out
tiles; ACT lane = bottom halves (rows 64-127 first, feeding pooled) +
odd out tiles; gpsimd SWDGE = border memsets first, then tiny gate
weights + expert weights B. Conv tiles dep on the whole XX tile, so
lanes carry as little as possible before the last x chunk. Pooled GAP
partials: top chunks 0,1 via ACT ACTIVATE+accum (interleaves with its
DMA queue), bottom chunks 2,3 on DVE; the gate matmul uses a
[wg1; wg1]-stacked lhsT to fold the cross-half sum. Gate math runs on
DVE except the softmax exp (ACT, ordered before the late ACT DMAs).
"""
import numpy as np
from contextlib import ExitStack

import concourse.bass as bass
import concourse.tile as tile
from concourse import bacc, mybir
from concourse.bass_utils import run_bass_kernel_spmd
from concourse.tile import add_dep_helper

F32 = mybir.dt.float32
F32R = mybir.dt.float32r
AX = mybir.AxisListType
OP = mybir.AluOpType
ACTF = mybir.ActivationFunctionType

B, C, H, W, E, GH = 16, 64, 128, 128, 8, 16
NCORES = 8
SPB = B // NCORES          # samples per core
HP, WP = H + 2, W + 2      # 130
FLAT = HP * WP             # 16900
NT = H // 4                # 32 conv tiles of 4 rows each
RB = 32                    # x-load chunk rows
GATE_SPLIT = 18            # emit sample-1 gate work after this many s0 conv tiles

_cache = {}


def _emit_borders(nc, XX):
    """Zero the padded borders (disjoint from the DMA-written interiors)."""
    nc.gpsimd.memset(XX[0:64, 0:130].bitcast(F32), 0.0)
    mid_top = XX[0:64, 130:16770].rearrange("p (r c) -> p r c", c=WP)
    nc.gpsimd.memset(mid_top[:, :, 0:1].bitcast(F32), 0.0)
    nc.gpsimd.memset(mid_top[:, :, 129:130].bitcast(F32), 0.0)
    nc.gpsimd.memset(XX[0:64, 16770:16900].bitcast(F32), 0.0)
    nc.gpsimd.memset(XX[64:128, 0:129].bitcast(F32), 0.0)
    mid_bot = XX[64:128, 257:16767].rearrange("p (r c) -> p r c", c=WP)
    nc.gpsimd.memset(mid_bot[:, :, 0:2].bitcast(F32), 0.0)
    nc.gpsimd.memset(XX[64:128, 16767:16900].bitcast(F32), 0.0)


def _emit_sample_loads(nc, pools, s, XX, xs_ap, mid_sp=None):
    """Input DMAs + pooled partial sums for sample s.

    SP lane (slower DGE): top chunks 0,1,2.
    ACT lane (faster DGE): bottom chunks 2,3 (rows 64-127, pooled
    inputs), bottom chunks 0,1, top chunk 3, then [mid_sp()].
    DVE sums top chunks 0,1 (partitions 0-63) and bottom chunks 2,3
    (partitions 64-127) as they land.
    """
    f = pools
    top_int = XX[0:64, 131:16771].rearrange("p (r c) -> p r c", c=WP)
    bot_int = XX[64:128, 129:16769].rearrange("p (r c) -> p r c", c=WP)

    def top(k, eng):
        eng.dma_start(
            top_int[:, RB * k : RB * (k + 1), 0:128],
            xs_ap[s, :, RB * k : RB * (k + 1), :],
        )

    def bot(k):
        return nc.scalar.dma_start(
            bot_int[:, RB * k : RB * (k + 1), 0:128],
            xs_ap[s, :, RB * k : RB * (k + 1), :],
        )

    part = f["gate"].tile([128, 2], F32, tag="part", name=f"part{s}")

    top(0, nc.sync)
    top(1, nc.sync)
    if mid_sp is not None:
        mid_sp()
    top(2, nc.sync)
    top(3, nc.sync)
    bot(2)
    bot(3)
    # ACT-lane compute: pooled partials of top chunks 0,1
    scrA = f["scratch"].tile([64, RB, 128], F32, tag="scrA", name=f"scrA{s}")
    for k in (0, 1):
        nc.scalar.activation(
            scrA[:],
            top_int[:, RB * k : RB * (k + 1), 0:128].bitcast(F32),
            ACTF.Copy,
            accum_out=part[0:64, k : k + 1],
        )
    bot_dmas = [bot(0), bot(1)]
    # DVE: pooled partials of bottom chunks 2,3 (partitions 64-127)
    def dve_part(view, dst):
        scrB = f["scratch"].tile([128, RB, 128], F32, tag="scrB", name=f"scrB{s}_{dst[0]}_{dst[1]}")
        nc.vector.tensor_scalar(
            scrB[dst[0] : dst[0] + 64],
            view.bitcast(F32),
            0.0,
            0.0,
            OP.add,
            OP.add,
            accum_out=part[dst[0] : dst[0] + 64, dst[1] : dst[1] + 1],
        )

    dve_part(bot_int[:, RB * 2 : RB * 3, 0:128], (64, 0))
    dve_part(bot_int[:, RB * 3 : RB * 4, 0:128], (64, 1))
    pooled = f["gate"].tile([128, 1], F32, tag="pooled", name=f"pooled{s}")
    nc.vector.tensor_reduce(pooled, part[:], axis=AX.X, op=OP.add)
    return pooled, bot_dmas


def _emit_sample_gate(nc, pools, s, pooled, consts):
    """Gate MLP + softmax + top-2 + combined weights/bias for one sample.

    Uses exp-without-max-sub (logits are small) and folds the top-2 mask
    and renormalization:  w8 = (u>=m2)*u / (sum((u>=m2)*u) + sum(u)*1e-8)
    which equals the reference's normalized-probs formula exactly.
    Returns (wcombr, b_comb, exp_inst).
    """
    f = pools
    g = f["gate"]
    wg1x2_sb, bg1_sb, wg2_sb, bg2_sb, bexp_sb, wpsA_sb, wpsB_sb, ones = consts
    n = lambda base: f"{base}{s}"

    h_ps = f["gpsum"].tile([GH, 1], F32, tag="gps", name=n("h_ps"))
    nc.tensor.matmul(h_ps[:], lhsT=wg1x2_sb[:], rhs=pooled[:], start=True, stop=True)
    # h_ext = [relu(pooled_sum @ (wg1/(H*W)) + bg1); 1.0] — the trailing 1.0
    # row turns bg2 into a wg2 row in the next matmul
    h_ext = g.tile([GH + 1, 1], F32, tag="h_ext", name=n("h_ext"))
    nc.gpsimd.dma_start(h_ext[GH : GH + 1, 0:1], ones[0:1, 0:1])
    nc.vector.tensor_scalar(
        h_ext[0:GH, :], h_ps[:], bg1_sb[:], 0.0, OP.add, OP.max
    )

    lg_ps = f["gpsum"].tile([1, E], F32, tag="gps", name=n("lg_ps"))
    nc.tensor.matmul(lg_ps[:], lhsT=h_ext[:], rhs=wg2_sb[:], start=True, stop=True)

    # u = exp(logits) (unnormalized softmax; |logits| is tiny, no max-sub)
    u = g.tile([1, E], F32, tag="u", name=n("u"))
    exp_inst = nc.scalar.activation(u[:], lg_ps[:], ACTF.Exp)
    usum = g.tile([1, 1], F32, tag="usum", name=n("usum"))
    nc.vector.tensor_reduce(usum[:], u[:], axis=AX.X, op=OP.add)
    # top-2: pm = (u < max)*u (valid since u>0), m2 = 2nd max, spv = (u>=m2)*u
    m1p = g.tile([1, 1], F32, tag="m1p", name=n("m1p"))
    nc.vector.tensor_reduce(m1p[:], u[:], axis=AX.X, op=OP.max)
    pm = g.tile([1, E], F32, tag="pm", name=n("pm"))
    nc.vector.scalar_tensor_tensor(pm[:], u[:], m1p[:], u[:], op0=OP.is_lt, op1=OP.mult)
    m2 = g.tile([1, 1], F32, tag="m2", name=n("m2"))
    nc.vector.tensor_reduce(m2[:], pm[:], axis=AX.X, op=OP.max)
    spv = g.tile([1, E], F32, tag="spv", name=n("spv"))
    nc.vector.scalar_tensor_tensor(spv[:], u[:], m2[:], u[:], op0=OP.is_ge, op1=OP.mult)
    dsum = g.tile([1, 1], F32, tag="dsum", name=n("dsum"))
    nc.vector.tensor_reduce(dsum[:], spv[:], axis=AX.X, op=OP.add)
    dd = g.tile([1, 1], F32, tag="dd", name=n("dd"))
    nc.vector.scalar_tensor_tensor(dd[:], usum[:], 1e-8, dsum[:], op0=OP.mult, op1=OP.add)
    rr = g.tile([1, 1], F32, tag="rr", name=n("rr"))
    nc.vector.reciprocal(rr[:], dd[:])
    w8 = g.tile([1, E], F32, tag="w8", name=n("w8"))
    nc.vector.tensor_scalar_mul(w8[:], spv[:], rr[:])

    # broadcast w8 down all 128 partitions: [128, E] = ones[1,128]^T @ w8[1,E]
    wb_ps = f["gpsum"].tile([128, E], F32, tag="wbps", name=n("wb_ps"), bufs=1)
    nc.tensor.matmul(wb_ps[:], lhsT=ones[:], rhs=w8[:], start=True, stop=True)
    wb128 = wb_ps

    # combined bias path (off critical path): b_comb = b_exp^T @ w8^T
    w8c_ps = f["gpsum"].tile([E, 1], F32, tag="gps", name=n("w8c_ps"))
    nc.tensor.matmul(w8c_ps[:], lhsT=w8[:], rhs=ones[:, 0:1], start=True, stop=True)
    w8col = g.tile([E, 1], F32, tag="w8col", name=n("w8col"))
    nc.vector.tensor_copy(w8col[:], w8c_ps[:])
    bc_ps = f["gpsum"].tile([C, 1], F32, tag="gps", name=n("bc_ps"))
    nc.tensor.matmul(bc_ps[:], lhsT=bexp_sb[:], rhs=w8col[:], start=True, stop=True)
    b_comb = g.tile([C, 1], F32, tag="b_comb", name=n("b_comb"))
    nc.vector.tensor_copy(b_comb[:], bc_ps[:])

    # combined conv weights: one fused MAC chain over [128, 6, C]
    # (slots 0-2 = paired dx taps, 3-5 = dx=0 taps w/ zero bottom rows)
    wcomb = f["wcomb"].tile([128, 6, C], F32, tag="wcomb", name=n("wcomb"))
    nc.vector.tensor_scalar_mul(wcomb[:], wpsA_sb[:, 0], wb128[:, 0:1])
    for e in range(1, E):
        src_w = wpsA_sb[:, e] if e < 4 else wpsB_sb[:, e - 4]
        nc.vector.scalar_tensor_tensor(
            wcomb[:], src_w, wb128[:, e : e + 1], wcomb[:],
            op0=OP.mult, op1=OP.add,
        )
    wcombr = f["wcomb"].tile([128, 6, C], F32R, tag="wcombr", name=n("wcombr"))
    nc.vector.tensor_copy(wcombr[:], wcomb[:])
    return wcombr, b_comb, exp_inst


def _emit_conv_tiles(nc, pools, s, XX, wcombr, b_comb, out_ap, t_range):
    """Conv tiles (4 output rows each) for sample s."""
    f = pools
    XX3 = XX[:, 0:FLAT].rearrange("p (r c) -> p r c", c=WP)
    for t in t_range:
        ps = f["cpsum"].tile([C, 4 * W], F32, tag="cps", name=f"cps{s}_{t}")
        r0 = 4 * t
        for dyi in range(3):
            nc.tensor.matmul(
                ps[:],
                lhsT=wcombr[:, dyi, :],
                rhs=XX3[:, r0 + dyi : r0 + dyi + 4, 0:128],
                start=(dyi == 0),
                stop=False,
            )
        for dyi in range(3):
            nc.tensor.matmul(
                ps[:],
                lhsT=wcombr[:, 3 + dyi, :],
                rhs=XX3[:, r0 + dyi : r0 + dyi + 4, 1:129],
                start=False,
                stop=(dyi == 2),
            )
        out_sb = f["stage"].tile([C, 4, W], F32, tag="stage", name=f"ost{s}_{t}")
        nc.vector.scalar_tensor_tensor(
            out_sb[:],
            ps[:].rearrange("p (a b) -> p a b", b=W),
            b_comb[:],
            XX3[0:64, r0 + 1 : r0 + 5, 1:129].bitcast(F32),
            op0=OP.add,
            op1=OP.add,
        )
        eng = nc.sync if t % 2 == 0 else nc.scalar
        eng.dma_start(out_ap[s, :, r0 : r0 + 4, :], out_sb[:])


def build_program():
    if "nc" in _cache:
        return _cache["nc"]
    nc = bacc.Bacc("TRN2", target_bir_lowering=False, debug=False, enable_asserts=False)
    xs_ap = nc.dram_tensor("xs", [SPB, C, H, W], F32R, kind="ExternalInput").ap()
    wpsA_d = nc.dram_tensor("wpsA", [128, E // 2, 6, C], F32, kind="ExternalInput").ap()
    wpsB_d = nc.dram_tensor("wpsB", [128, E // 2, 6, C], F32, kind="ExternalInput").ap()
    wg1_d = nc.dram_tensor("wg1", [128, GH], F32, kind="ExternalInput").ap()
    bg1_d = nc.dram_tensor("bg1", [GH, 1], F32, kind="ExternalInput").ap()
    wg2_d = nc.dram_tensor("wg2", [GH + 1, E], F32, kind="ExternalInput").ap()
    bg2_d = nc.dram_tensor("bg2", [1, E], F32, kind="ExternalInput").ap()
    bexp_d = nc.dram_tensor("b_exp", [E, C], F32, kind="ExternalInput").ap()
    out_ap = nc.dram_tensor("out", [SPB, C, H, W], F32, kind="ExternalOutput").ap()

    with tile.TileContext(nc) as tc, ExitStack() as ctx:
        pools = {
            "const": ctx.enter_context(tc.tile_pool(name="const", bufs=1)),
            "xx": ctx.enter_context(tc.tile_pool(name="xx", bufs=SPB)),
            "gate": ctx.enter_context(tc.tile_pool(name="gate", bufs=2)),
            "wcomb": ctx.enter_context(tc.tile_pool(name="wcomb", bufs=2)),
            "stage": ctx.enter_context(tc.tile_pool(name="stage", bufs=6)),
            "scratch": ctx.enter_context(tc.tile_pool(name="scratch", bufs=1)),
            "gpsum": ctx.enter_context(tc.tile_pool(name="gpsum", bufs=1, space="PSUM")),
            "cpsum": ctx.enter_context(tc.tile_pool(name="cpsum", bufs=6, space="PSUM")),
        }
        cp = pools["const"]
        # XX tiles + their border memsets first on gpsimd, so the memsets
        # never delay the x-chunk DMAs whose regions they border
        XX0 = pools["xx"].tile([128, FLAT], F32R, tag="XX", name="XX0")
        XX1 = pools["xx"].tile([128, FLAT], F32R, tag="XX", name="XX1")
        _emit_borders(nc, XX0)
        _emit_borders(nc, XX1)
        ones = cp.tile([1, 128], F32)
        nc.gpsimd.memset(ones[:], 1.0)
        # prewarm the ACT exp table before the ACT lane fills with DMAs
        warm = cp.tile([1, 1], F32)
        nc.scalar.activation(warm[:], ones[:, 0:1], ACTF.Exp)
        # tiny gate weights + expert weights B on the gpsimd SWDGE lane
        wg1x2_sb = cp.tile([128, GH], F32)
        nc.gpsimd.dma_start(wg1x2_sb[:], wg1_d[:])
        bg1_sb = cp.tile([GH, 1], F32)
        nc.gpsimd.dma_start(bg1_sb[:], bg1_d[:])
        wg2_sb = cp.tile([GH + 1, E], F32)
        nc.gpsimd.dma_start(wg2_sb[:], wg2_d[:])
        bg2_sb = cp.tile([1, E], F32)
        nc.gpsimd.dma_start(bg2_sb[:], bg2_d[:])
        bexp_sb = cp.tile([E, C], F32)
        nc.gpsimd.dma_start(bexp_sb[:], bexp_d[:])
        wpsA_sb = cp.tile([128, E // 2, 6, C], F32)
        wpsB_sb = cp.tile([128, E // 2, 6, C], F32)
        nc.gpsimd.dma_start(wpsB_sb[:], wpsB_d[:])

        def load_wpsA():
            nc.sync.dma_start(wpsA_sb[:], wpsA_d[:])

        pooled0, bots0 = _emit_sample_loads(nc, pools, 0, XX0, xs_ap, mid_sp=load_wpsA)
        consts = (wg1x2_sb, bg1_sb, wg2_sb, bg2_sb, bexp_sb, wpsA_sb, wpsB_sb, ones)

        g0 = _emit_sample_gate(nc, pools, 0, pooled0, consts)
        add_dep_helper(bots0[0].ins, g0[2].ins, sync=False,
                       reason="s0 late bottom DMAs after s0 softmax exp")
        pooled1, bots1 = _emit_sample_loads(nc, pools, 1, XX1, xs_ap)

        _emit_conv_tiles(nc, pools, 0, XX0, *g0[:2], out_ap, range(0, GATE_SPLIT))
        g1 = _emit_sample_gate(nc, pools, 1, pooled1, consts)
        _emit_conv_tiles(nc, pools, 0, XX0, *g0[:2], out_ap, range(GATE_SPLIT, NT))
        _emit_conv_tiles(nc, pools, 1, XX1, *g1[:2], out_ap, range(0, NT))

    nc.compile()
    _cache["nc"] = nc
    return nc


def _round_fp32r(a):
    """Round fp32 array to the fp32r grid (RNE to 11-bit mantissa, low 12
    bits of the fp32 word zeroed) — what the PE consumes in fp32r mode."""
    u = np.ascontiguousarray(a, dtype=np.float32).view(np.uint32)
    r = (u + np.uint32(0x7FF) + ((u >> np.uint32(12)) & np.uint32(1))) & np.uint32(
        0xFFFFF000
    )
    return r.view(np.float32)


def host_prep(x, wg1, bg1, wg2, bg2, w_exp, b_exp):
    """Host-side layout prep + per-core sharding. Returns in_maps list."""
    x = _round_fp32r(np.asarray(x, dtype=np.float32))
    wg1 = np.asarray(wg1, dtype=np.float32)
    bg1 = np.asarray(bg1, dtype=np.float32).reshape(GH, 1)
    wg2 = np.asarray(wg2, dtype=np.float32)
    bg2 = np.asarray(bg2, dtype=np.float32).reshape(1, E)
    w_exp = np.asarray(w_exp, dtype=np.float32)
    b_exp = np.asarray(b_exp, dtype=np.float32)

    # w_exp [E, O, I, KH, KW] -> wt [I, E, KH, KW, O]
    wt = np.transpose(w_exp, (2, 0, 3, 4, 1))
    # paired taps: top partitions = dx=-1, bottom = dx=+1
    wpair = np.concatenate([wt[:, :, :, 0, :], wt[:, :, :, 2, :]], axis=0)
    # single taps: dx=0 on top, zeros on bottom
    wsing = np.concatenate([wt[:, :, :, 1, :], np.zeros_like(wt[:, :, :, 1, :])], axis=0)
    # merged [128, E, 6, O]: slots 0-2 pairs, 3-5 singles
    wps = np.concatenate([wpair, wsing], axis=2)

    shared = {
        "wpsA": np.ascontiguousarray(wps[:, 0:4]),
        "wpsB": np.ascontiguousarray(wps[:, 4:8]),
        "wg1": np.ascontiguousarray(np.concatenate([wg1, wg1], axis=0) / (H * W)),
        "bg1": np.ascontiguousarray(bg1),
        "wg2": np.ascontiguousarray(np.concatenate([wg2, bg2], axis=0)),
        "bg2": np.ascontiguousarray(bg2),
        "b_exp": np.ascontiguousarray(b_exp),
    }
    return [
        {"xs": np.ascontiguousarray(x[SPB * k : SPB * (k + 1)]), **shared}
        for k in range(NCORES)
    ]


def kernel(x, wg1, bg1, wg2, bg2, w_exp, b_exp):
    nc = build_program()
    in_maps = host_prep(x, wg1, bg1, wg2, bg2, w_exp, b_exp)
    res = run_bass_kernel_spmd(nc, in_maps, list(range(NCORES)))
    return np.concatenate([res.results[k]["out"] for k in range(NCORES)], axis=0)



# revision 15
# speedup vs baseline: 1.0094x; 1.0094x over previous
"""MoE block (top-2 routed 3x3 conv experts) Trainium2 Bass kernel — v2.

Data-parallel over batch, 2 samples per core on 8 cores. The conv is
linear in the kernel, so the top-2 expert kernels are combined with the
routing probabilities first (w_comb = sum_e p_e W_e + I, the +I folding
the residual into the center tap), then one 3x3 SAME conv per sample.

Conv-as-matmul, 3 matmuls per 3-row tile (vs 6 in v1): x is stored
zero-padded [130x130] per channel in SBUF (bf16), partitions 0-63 =
channels, partitions 64-127 = same channels shifted +2 pixels. lhsT is
[128K, 128M]: M 0:64 ("A") holds taps dx=0 (top K) and dx=2 (bottom K);
M 64:128 ("B") holds the center dx=1 taps (top K, bottom zero). Each of
the 3 dy reads [128, 3rows, 130cols] (390 cols >= 256 keeps 1 col/cyc)
accumulates into one PSUM bank; all 4 tap-groups land in one pass.
The DVE post-op then computes out = psA[j] + psB[j+1] + b_comb in a
single scalar_tensor_tensor per 2-tile pair (psB read at partition base
64, column offset +1), written as bf16.

Everything on the DMA path is bf16 (x in, weights, out), halving HBM
traffic vs v1; gate math stays f32. Host upcasts the bf16 output.

DMA/engine lanes: SP = x top chunks + even out pairs; ACT = x bottom
chunks + s1 GAP + exp; DVE queue = wps weights + s0 late bottoms, DVE
compute = s0 GAP, gate vector ops, wcA MAC chains, conv post-ops;
gpsimd = border memsets, gate consts, x top chunks 2/3, wcB MAC
chains, odd out pairs.
"""
import numpy as np
from contextlib import ExitStack

import ml_dtypes

import concourse.bass as bass
import concourse.tile as tile
from concourse import bacc, mybir
from concourse.bass_utils import run_bass_kernel_spmd

F32 = mybir.dt.float32
BF16 = mybir.dt.bfloat16
AX = mybir.AxisListType
OP = mybir.AluOpType
ACTF = mybir.ActivationFunctionType

B, C, H, W, E, GH = 16, 64, 128, 128, 8, 16
NCORES = 8
SPB = B // NCORES          # samples per core
HP, WP = H + 2, W + 2      # 130
FLAT = HP * WP             # 16900
RB = 32                    # x-load chunk rows
NT = 43                    # 42 3-row tiles + 1 2-row tile
TILES = [(3 * t, 3) for t in range(42)] + [(126, 2)]
PAIRS = [(2 * p, 2 * p + 1) for p in range(21)] + [(42,)]
GATE_SPLIT = 10            # emit s1 gate work after this many s0 pairs

NPBF16 = ml_dtypes.bfloat16

_cache = {}


def _emit_borders(nc, XX):
    """Zero the padded borders (disjoint from the DMA-written interiors)."""
    nc.gpsimd.memset(XX[0:64, 0:130], 0.0)
    mid_top = XX[0:64, 130:16770].rearrange("p (r c) -> p r c", c=WP)
    nc.gpsimd.memset(mid_top[:, :, 0:1], 0.0)
    nc.gpsimd.memset(mid_top[:, :, 129:130], 0.0)
    nc.gpsimd.memset(XX[0:64, 16770:16900], 0.0)
    nc.gpsimd.memset(XX[64:128, 0:129], 0.0)
    mid_bot = XX[64:128, 257:16767].rearrange("p (r c) -> p r c", c=WP)
    nc.gpsimd.memset(mid_bot[:, :, 0:2], 0.0)
    nc.gpsimd.memset(XX[64:128, 16767:16900], 0.0)


def _views(XX):
    top_int = XX[0:64, 131:16771].rearrange("p (r c) -> p r c", c=WP)
    bot_int = XX[64:128, 129:16769].rearrange("p (r c) -> p r c", c=WP)
    return top_int, bot_int


def _emit_loads(nc, s, XX, xs_ap, lanes):
    """Input DMAs for sample s. lanes = dict chunk->engine for top/bot."""
    top_int, bot_int = _views(XX)
    for k in range(4):
        lanes["top"][k].dma_start(
            top_int[:, RB * k : RB * (k + 1), 0:128],
            xs_ap[s, :, RB * k : RB * (k + 1), :],
        )
    for k in lanes["bot_order"]:
        lanes["bot"][k].dma_start(
            bot_int[:, RB * k : RB * (k + 1), 0:128],
            xs_ap[s, :, RB * k : RB * (k + 1), :],
        )


def _emit_gap_chunks(nc, pools, s, XX, part, plan):
    """GAP partial sums: top chunks 0,1 on partitions 0:64, bottom chunks
    2,3 on partitions 64:128 (the [wg1; wg1]-stacked gate lhsT folds the
    cross-half sum). plan = list of (is_bot, k, slot, engine)."""
    f = pools
    top_int, bot_int = _views(XX)
    for is_bot, k, slot, eng in plan:
        if is_bot:
            src = bot_int[:, RB * (2 + k) : RB * (3 + k), 0:128]
            dst_lo, dst_hi = 64, 128
        else:
            src = top_int[:, RB * k : RB * (k + 1), 0:128]
            dst_lo, dst_hi = 0, 64
        acc = part[dst_lo:dst_hi, slot : slot + 1]
        if eng == "act":
            nc.scalar.activation(
                f["scrS"][dst_lo:dst_hi], src, ACTF.Copy, accum_out=acc
            )
        else:
            nc.vector.tensor_scalar(
                f["scrD"][dst_lo:dst_hi], src, 0.0, 0.0, OP.add, OP.add,
                accum_out=acc,
            )


def _emit_gate(nc, pools, s, pooled, consts):
    """Gate MLP + softmax + top-2 for one sample (all f32).

    exp-without-max-sub (logits are small); folds the top-2 mask and
    renormalization: w8 = (u>=m2)*u / (sum((u>=m2)*u) + sum(u)*1e-8).
    Returns (wb_sb [128,E] f32 per-partition probs, b_comb [C,1]).
    """
    f = pools
    g = f["gate"]
    wg1x2_sb, bg1_sb, wg2_sb, bexp_sb, ones = consts
    n = lambda base: f"{base}{s}"

    h_ps = f["gpsum"].tile([GH, 1], F32, tag="gps", name=n("h_ps"))
    nc.tensor.matmul(h_ps[:], lhsT=wg1x2_sb, rhs=pooled[:], start=True, stop=True)
    h_ext = g.tile([GH + 1, 1], F32, tag="h_ext", name=n("h_ext"))
    nc.gpsimd.dma_start(h_ext[GH : GH + 1, 0:1], ones[0:1, 0:1])
    nc.vector.tensor_scalar(h_ext[0:GH, :], h_ps[:], bg1_sb, 0.0, OP.add, OP.max)

    lg_ps = f["gpsum"].tile([1, E], F32, tag="gps", name=n("lg_ps"))
    nc.tensor.matmul(lg_ps[:], lhsT=h_ext[:], rhs=wg2_sb, start=True, stop=True)

    u = g.tile([1, E], F32, tag="u", name=n("u"))
    nc.scalar.activation(u[:], lg_ps[:], ACTF.Exp)
    usum = g.tile([1, 1], F32, tag="usum", name=n("usum"))
    nc.vector.tensor_reduce(usum[:], u[:], axis=AX.X, op=OP.add)
    m1p = g.tile([1, 1], F32, tag="m1p", name=n("m1p"))
    nc.vector.tensor_reduce(m1p[:], u[:], axis=AX.X, op=OP.max)
    pm = g.tile([1, E], F32, tag="pm", name=n("pm"))
    nc.vector.scalar_tensor_tensor(pm[:], u[:], m1p[:], u[:], op0=OP.is_lt, op1=OP.mult)
    m2 = g.tile([1, 1], F32, tag="m2", name=n("m2"))
    nc.vector.tensor_reduce(m2[:], pm[:], axis=AX.X, op=OP.max)
    spv = g.tile([1, E], F32, tag="spv", name=n("spv"))
    nc.vector.scalar_tensor_tensor(spv[:], u[:], m2[:], u[:], op0=OP.is_ge, op1=OP.mult)
    dsum = g.tile([1, 1], F32, tag="dsum", name=n("dsum"))
    nc.vector.tensor_reduce(dsum[:], spv[:], axis=AX.X, op=OP.add)
    dd = g.tile([1, 1], F32, tag="dd", name=n("dd"))
    nc.vector.scalar_tensor_tensor(dd[:], usum[:], 1e-8, dsum[:], op0=OP.mult, op1=OP.add)
    rr = g.tile([1, 1], F32, tag="rr", name=n("rr"))
    nc.vector.reciprocal(rr[:], dd[:])
    w8 = g.tile([1, E], F32, tag="w8", name=n("w8"))
    nc.vector.tensor_scalar_mul(w8[:], spv[:], rr[:])

    # broadcast w8 down all 128 partitions, then stage to SBUF for MACs
    wb_ps = f["gpsum"].tile([128, E], F32, tag="gps", name=n("wb_ps"))
    nc.tensor.matmul(wb_ps[:], lhsT=ones[:], rhs=w8[:], start=True, stop=True)
    wb_sb = g.tile([128, E], F32, tag="wb_sb", name=n("wb_sb"))
    nc.vector.tensor_copy(wb_sb[:], wb_ps[:])

    # combined bias: b_comb = b_exp^T @ w8^T
    w8c_ps = f["gpsum"].tile([E, 1], F32, tag="gps", name=n("w8c_ps"))
    nc.tensor.matmul(w8c_ps[:], lhsT=w8[:], rhs=ones[:, 0:1], start=True, stop=True)
    w8col = g.tile([E, 1], F32, tag="w8col", name=n("w8col"))
    nc.vector.tensor_copy(w8col[:], w8c_ps[:])
    bc_ps = f["gpsum"].tile([C, 1], F32, tag="gps", name=n("bc_ps"))
    nc.tensor.matmul(bc_ps[:], lhsT=bexp_sb, rhs=w8col[:], start=True, stop=True)
    b_comb = g.tile([C, 1], F32, tag="b_comb", name=n("b_comb"))
    nc.vector.tensor_copy(b_comb[:], bc_ps[:])
    return wb_sb, b_comb


def _emit_mac(nc, pools, s, wb_sb, wpsA_sb, wpsB_sb):
    """wcomb = sum_e p_e wps_e: single DVE MAC chain accumulating in bf16
    (all-16-bit operands keep the DVE 2x mode; gpsimd lacks TSP support).
    Residual identity is pre-folded into every expert's center-tap B-half
    on the host."""
    f = pools
    wcombr = f["wcomb"].tile([128, 3, 128], BF16, tag="wcombr", name=f"wcombr{s}")
    nc.vector.tensor_scalar_mul(wcombr[:], wpsA_sb[:, 0], wb_sb[:, 0:1])
    for e in range(1, E):
        src = wpsA_sb[:, e] if e < 4 else wpsB_sb[:, e - 4]
        nc.vector.scalar_tensor_tensor(
            wcombr[:], src, wb_sb[:, e : e + 1], wcombr[:],
            op0=OP.mult, op1=OP.add,
        )
    return wcombr


def _emit_pair(nc, pools, s, p, XX, wcombr, b_comb, obuf, ocol):
    """Conv for tile pair p: 6 (or 3) matmuls into a 2-bank PSUM tile.
    The DVE post-op may read only ONE operand from PSUM (NCC_IBVF027),
    so ACT first stages the B half (shifted +1 col) to SBUF as bf16,
    then DVE combines psA + b_comb + sbB into the out batch buffer."""
    f = pools
    XX3 = XX[:, 0:FLAT].rearrange("p (r c) -> p r c", c=WP)
    tl = PAIRS[p]
    ps = f["cpsum"].tile([128, 2, 512], F32, tag="cps", name=f"cps{s}_{p}")
    for t01, t in enumerate(tl):
        r0, nr = TILES[t]
        for dyi in range(3):
            nc.tensor.matmul(
                ps[:, t01, 0 : nr * WP],
                lhsT=wcombr[:, dyi, :],
                rhs=XX3[:, r0 + dyi : r0 + dyi + nr, :],
                start=(dyi == 0),
                stop=(dyi == 2),
            )
    # post-op in flat 390-cols-per-tile layout (stt APs are limited to
    # partition + 2 free dims); the junk pad columns (c=128,129 of each
    # 130-block) are skipped later by the out-DMA's strided read
    if len(tl) == 2:
        sbB = f["stage"].tile([128, 2, 390], BF16, tag="sbB", name=f"sbB{s}_{p}")
        nc.scalar.activation(sbB[0:64], ps[64:128, :, 1:391], ACTF.Copy)
        nc.vector.scalar_tensor_tensor(
            obuf[:, ocol : ocol + 780].rearrange("p (t c) -> p t c", c=390),
            ps[0:64, :, 0:390],
            b_comb[:],
            sbB[0:64],
            op0=OP.add,
            op1=OP.add,
        )
    else:
        ncols = TILES[tl[0]][1] * WP
        sbB = f["stage"].tile([128, 2, 390], BF16, tag="sbB", name=f"sbB{s}_{p}")
        nc.scalar.activation(sbB[0:64, 0, 0:ncols], ps[64:128, 0, 1 : 1 + ncols], ACTF.Copy)
        nc.vector.scalar_tensor_tensor(
            obuf[:, ocol : ocol + ncols],
            ps[0:64, 0, 0:ncols],
            b_comb[:],
            sbB[0:64, 0, 0:ncols],
            op0=OP.add,
            op1=OP.add,
        )


def build_program():
    if "nc" in _cache:
        return _cache["nc"]
    nc = bacc.Bacc("TRN2", target_bir_lowering=False, debug=False, enable_asserts=False)
    xs_ap = nc.dram_tensor("xs", [SPB, C, H, W], BF16, kind="ExternalInput").ap()
    wpsA_d = nc.dram_tensor("wpsA", [128, E // 2, 3, 128], BF16, kind="ExternalInput").ap()
    wpsB_d = nc.dram_tensor("wpsB", [128, E // 2, 3, 128], BF16, kind="ExternalInput").ap()
    gconst_d = nc.dram_tensor("gconst", [128, 90], F32, kind="ExternalInput").ap()
    out_ap = nc.dram_tensor("out", [SPB, C, H, W], BF16, kind="ExternalOutput").ap()

    with tile.TileContext(nc) as tc, ExitStack() as ctx:
        pools = {
            "const": ctx.enter_context(tc.tile_pool(name="const", bufs=1)),
            "xx": ctx.enter_context(tc.tile_pool(name="xx", bufs=SPB)),
            "gate": ctx.enter_context(tc.tile_pool(name="gate", bufs=2)),
            "wcomb": ctx.enter_context(tc.tile_pool(name="wcomb", bufs=2)),
            "stage": ctx.enter_context(tc.tile_pool(name="stage", bufs=6)),
            "gpsum": ctx.enter_context(tc.tile_pool(name="gpsum", bufs=1, space="PSUM")),
            "cpsum": ctx.enter_context(tc.tile_pool(name="cpsum", bufs=3, space="PSUM")),
        }
        cp = pools["const"]
        XX0 = pools["xx"].tile([128, FLAT], BF16, tag="XX", name="XX0")
        XX1 = pools["xx"].tile([128, FLAT], BF16, tag="XX", name="XX1")
        # border memsets first on gpsimd so they never delay x-chunk DMAs
        _emit_borders(nc, XX0)
        gconst_sb = cp.tile([128, 90], F32)
        nc.gpsimd.dma_start(gconst_sb[:], gconst_d[:])
        _emit_borders(nc, XX1)
        ones = cp.tile([1, 128], F32)
        nc.gpsimd.memset(ones[:], 1.0)
        # prewarm the ACT exp table before the ACT lane fills with DMAs
        warm = cp.tile([1, 1], F32)
        nc.scalar.activation(warm[:], ones[:, 0:1], ACTF.Exp)
        wpsA_sb = cp.tile([128, E // 2, 3, 128], BF16)
        wpsB_sb = cp.tile([128, E // 2, 3, 128], BF16)
        pools["scrD"] = cp.tile([128, RB, 128], BF16, name="scrD")
        pools["scrS"] = cp.tile([128, RB, 128], BF16, name="scrS")

        wg1x2_sb = gconst_sb[:, 0:16]
        bg1_sb = gconst_sb[0:16, 16:17]
        wg2_sb = gconst_sb[0:17, 17:25]
        bexp_sb = gconst_sb[0:8, 25:89]
        consts = (wg1x2_sb, bg1_sb, wg2_sb, bexp_sb, ones)

        lanes = {
            "top": {0: nc.sync, 1: nc.sync, 2: nc.gpsimd, 3: nc.gpsimd},
            "bot": {0: nc.scalar, 1: nc.scalar, 2: nc.scalar, 3: nc.scalar},
            "bot_order": [2, 3, 0, 1],
        }

        def emit_sample_pairs(s, XX, wcombr, bcomb, rng, hook=None):
            """Emit conv pairs; out rows accumulate in a flat 130-col-per-
            row batch buffer, drained 4 pairs (24 rows) per DMA on the
            gpsimd SWDGE lane. hook(p) emits interleaved work."""
            for p in rng:
                ob, orow, r0, nrows = obatch[s]
                if ob is None:
                    nrows = 24 if p + 4 <= 21 else (128 - 24 * 5)
                    r0 = TILES[PAIRS[p][0]][0]
                    ob = pools["stage"].tile(
                        [64, nrows * WP], BF16, tag="obuf", name=f"ob{s}_{p}",
                        bufs=3,
                    )
                    obatch[s] = [ob, 0, r0, nrows]
                    orow = 0
                _emit_pair(nc, pools, s, p, XX, wcombr, bcomb, ob, orow * WP)
                obatch[s][1] = orow = orow + sum(TILES[t][1] for t in PAIRS[p])
                if orow == nrows:
                    src = ob[:].rearrange("p (r c) -> p r c", c=WP)
                    nc.gpsimd.dma_start(
                        out_ap[s, :, r0 : r0 + nrows, :], src[:, :, 0:128]
                    )
                    obatch[s] = [None, 0, 0, 0]
                if hook is not None:
                    hook(p)

        obatch = {0: [None, 0, 0, 0], 1: [None, 0, 0, 0]}

        _emit_loads(nc, 0, XX0, xs_ap, lanes)
        # expert weights ride the SP lane behind s0's GAP-feeding chunks
        nc.sync.dma_start(wpsA_sb[:], wpsA_d[:])
        nc.sync.dma_start(wpsB_sb[:], wpsB_d[:])
        part0 = pools["gate"].tile([128, 2], F32, tag="part", name="part0")
        _emit_gap_chunks(
            nc, pools, 0, XX0, part0,
            [(0, 0, 0, "dve"), (1, 0, 0, "dve"), (0, 1, 1, "dve"), (1, 1, 1, "dve")],
        )
        pooled0 = pools["gate"].tile([128, 1], F32, tag="pooled", name="pooled0")
        nc.vector.tensor_reduce(pooled0, part0[:], axis=AX.X, op=OP.add)
        wb0, bcomb0 = _emit_gate(nc, pools, 0, pooled0, consts)
        wcombr0 = _emit_mac(nc, pools, 0, wb0, wpsA_sb, wpsB_sb)
        _emit_loads(nc, 1, XX1, xs_ap, lanes)

        part1 = pools["gate"].tile([128, 2], F32, tag="part", name="part1")

        def s1_gap_hook(p):
            # s1 GAP chunks on DVE, slipped between s0 post-ops as the
            # s1 x chunks land
            if p == 2:
                _emit_gap_chunks(
                    nc, pools, 1, XX1, part1,
                    [(0, 0, 0, "dve"), (1, 0, 0, "dve")],
                )
            elif p == 5:
                _emit_gap_chunks(
                    nc, pools, 1, XX1, part1,
                    [(1, 1, 1, "dve"), (0, 1, 1, "dve")],
                )

        emit_sample_pairs(0, XX0, wcombr0, bcomb0, range(GATE_SPLIT), s1_gap_hook)
        pooled1 = pools["gate"].tile([128, 1], F32, tag="pooled", name="pooled1")
        nc.vector.tensor_reduce(pooled1, part1[:], axis=AX.X, op=OP.add)
        wb1, bcomb1 = _emit_gate(nc, pools, 1, pooled1, consts)
        wcombr1 = _emit_mac(nc, pools, 1, wb1, wpsA_sb, wpsB_sb)
        emit_sample_pairs(0, XX0, wcombr0, bcomb0, range(GATE_SPLIT, len(PAIRS)))
        emit_sample_pairs(1, XX1, wcombr1, bcomb1, range(len(PAIRS)))

    nc.compile()
    _cache["nc"] = nc
    return nc


def host_prep(x, wg1, bg1, wg2, bg2, w_exp, b_exp):
    """Host-side layout prep + per-core sharding. Returns in_maps list."""
    x = np.asarray(x, dtype=np.float32).astype(NPBF16)
    wg1 = np.asarray(wg1, dtype=np.float32)
    bg1 = np.asarray(bg1, dtype=np.float32)
    wg2 = np.asarray(wg2, dtype=np.float32)
    bg2 = np.asarray(bg2, dtype=np.float32)
    w_exp = np.asarray(w_exp, dtype=np.float32)
    b_exp = np.asarray(b_exp, dtype=np.float32)

    # wps [128, E, 3(dy), 128]: K top/bottom = taps dx 0/2 on M 0:64 (A),
    # center dx=1 on M 64:128 top (B, bottom zero). Residual identity is
    # folded into every expert's center tap (sum of probs is ~1).
    wt = np.transpose(w_exp, (2, 0, 3, 4, 1))  # [I, E, dy, dx, O]
    wps = np.zeros((128, E, 3, 128), np.float32)
    wps[0:64, :, :, 0:64] = wt[:, :, :, 0, :]
    wps[64:128, :, :, 0:64] = wt[:, :, :, 2, :]
    wps[0:64, :, :, 64:128] = wt[:, :, :, 1, :]
    ii = np.arange(64)
    wps[ii, :, 1, 64 + ii] += 1.0

    gconst = np.zeros((128, 90), np.float32)
    gconst[:, 0:16] = np.concatenate([wg1, wg1], axis=0) / (H * W)
    gconst[0:16, 16] = bg1
    gconst[0:16, 17:25] = wg2
    gconst[16, 17:25] = bg2
    gconst[0:8, 25:89] = b_exp

    shared = {
        "wpsA": np.ascontiguousarray(wps[:, 0:4]).astype(NPBF16),
        "wpsB": np.ascontiguousarray(wps[:, 4:8]).astype(NPBF16),
        "gconst": gconst,
    }
    return [
        {"xs": np.ascontiguousarray(x[SPB * k : SPB * (k + 1)]), **shared}
        for k in range(NCORES)
    ]


def kernel(x, wg1, bg1, wg2, bg2, w_exp, b_exp):
    nc = build_program()
    in_maps = host_prep(x, wg1, bg1, wg2, bg2, w_exp, b_exp)
    res = run_bass_kernel_spmd(nc, in_maps, list(range(NCORES)))
    return np.concatenate(
        [np.asarray(res.results[k]["out"]).astype(np.float32) for k in range(NCORES)],
        axis=0,
    )


# revision 16
# speedup vs baseline: 1.3180x; 1.3057x over previous
"""MoE block (top-2 routed 3x3 conv experts) Trainium2 Bass kernel — v3.

Data-parallel over batch, 2 samples per core on 8 cores. The conv is
linear in the kernel, so the top-2 expert kernels are combined with the
routing probabilities first (w_comb = sum_e p_e W_e + I, the +I folding
the residual into the center tap), then one 3x3 SAME conv per sample.

Conv-as-matmul, 3 matmuls per 3-row tile: x lives zero-padded
[130x130] per channel in SBUF (bf16), partitions 0-63 = channels,
partitions 64-127 = same channels shifted +2 pixels. lhsT is
[128K, 128M]: M 0:64 ("A") = taps dx=0 (top K) / dx=2 (bottom K);
M 64:128 ("B") = center dx=1 taps (top K, bottom zero). The 3 dy reads
[128, 3rows x 130cols] (390 cols) accumulate into one PSUM bank.
Post-op: ACT stages psB (+1 col shift) to SBUF bf16 (the DVE may read
only ONE operand from PSUM), then one DVE scalar_tensor_tensor per
2-tile pair: out = psA + b_comb + sbB, written bf16 into a flat
130-col-per-row batch buffer.

DMA layout is the whole game (queues are descriptor-rate-bound at
~4.3ns/desc): the host ships x PRE-PADDED as [C, 16902] bf16 so every
x chunk is one contiguous descriptor per partition (64 descs vs 2048),
the bottom +2-shifted copy is the same buffer read at +2 elements, and
the out tensor is written padded [C, 128*130] (host strips). No border
memsets needed — pad zeros come from the host.

Lanes: SP = s0 top chunks, wps weights; ACT = bottom chunks (GAP-
feeding chunks of both samples first), s0-top GAP, exp, B-half
stagings; gpsimd = gconst, s1 top chunks, h_ext, out DMAs; DVE = s0-bot
GAP, gates, MAC chains, s1 GAP (spread between pair post-ops), pair
combines.
"""
import numpy as np
from contextlib import ExitStack

import ml_dtypes

import concourse.bass as bass
import concourse.tile as tile
from concourse import bacc, mybir
from concourse.bass_utils import run_bass_kernel_spmd

F32 = mybir.dt.float32
BF16 = mybir.dt.bfloat16
AX = mybir.AxisListType
OP = mybir.AluOpType
ACTF = mybir.ActivationFunctionType

B, C, H, W, E, GH = 16, 64, 128, 128, 8, 16
NCORES = 8
SPB = B // NCORES          # samples per core
HP, WP = H + 2, W + 2      # 130
FLAT = HP * WP             # 16900
QC = FLAT // 4             # x-load chunk size (4225 flat elements)
TILES = [(3 * t, 3) for t in range(42)] + [(126, 2)]
PAIRS = [(2 * p, 2 * p + 1) for p in range(21)] + [(42,)]
GATE_SPLIT = 13            # emit s1 gate work after this many s0 pairs

NPBF16 = ml_dtypes.bfloat16

_cache = {}


def _emit_loads(nc, s, XX, xs_ap, top_eng, bot_eng, bot_order):
    """Contiguous flat chunk DMAs: top half = xs[s,:,q], bottom half =
    the same bytes at +2 elements (builds the shifted copy for free)."""
    for q in range(4):
        top_eng.dma_start(
            XX[0:64, QC * q : QC * (q + 1)], xs_ap[s, :, QC * q : QC * (q + 1)]
        )
    for q in bot_order:
        bot_eng.dma_start(
            XX[64:128, QC * q : QC * (q + 1)],
            xs_ap[s, :, QC * q + 2 : QC * (q + 1) + 2],
        )


# GAP windows over the flat layout (pad zeros included, so plain flat
# ranges): top covers x rows 0..63 (+ the first elem of x row 64 that
# the shifted bottom copy misses), bottom covers x rows 64..127.
GAP_TOP = [(0, QC, 0), (QC, 2 * QC + 2, 1)]
GAP_BOT = [(2 * QC, 3 * QC, 0), (3 * QC, FLAT, 1)]


def _emit_gap_op(nc, pools, XX, part, win, is_bot, eng):
    a, b, slot = win
    if is_bot:
        src = XX[64:128, a:b]
        dst = pools["scrS" if eng == "act" else "scrD"][64:128]
        acc = part[64:128, slot : slot + 1]
    else:
        src = XX[0:64, a:b]
        dst = pools["scrS" if eng == "act" else "scrD"][0:64]
        acc = part[0:64, slot : slot + 1]
    if eng == "act":
        nc.scalar.activation(dst[:, 0 : b - a], src, ACTF.Copy, accum_out=acc)
    else:
        nc.vector.tensor_scalar(
            dst[:, 0 : b - a], src, 0.0, 0.0, OP.add, OP.add, accum_out=acc
        )


def _emit_gate(nc, pools, s, pooled, consts):
    """Gate MLP + softmax + top-2 for one sample (all f32).

    exp-without-max-sub (logits are small); folds the top-2 mask and
    renormalization: w8 = (u>=m2)*u / (sum((u>=m2)*u) + sum(u)*1e-8).
    Returns (wb_sb [128,E] f32 per-partition probs, b_comb [C,1]).
    """
    f = pools
    g = f["gate"]
    wg1x2_sb, bg1_sb, wg2_sb, bexp_sb, ones = consts
    n = lambda base: f"{base}{s}"

    h_ps = f["gpsum"].tile([GH, 1], F32, tag="gps", name=n("h_ps"))
    nc.tensor.matmul(h_ps[:], lhsT=wg1x2_sb, rhs=pooled[:], start=True, stop=True)
    h_ext = g.tile([GH + 1, 1], F32, tag="h_ext", name=n("h_ext"))
    nc.gpsimd.dma_start(h_ext[GH : GH + 1, 0:1], ones[0:1, 0:1])
    nc.vector.tensor_scalar(h_ext[0:GH, :], h_ps[:], bg1_sb, 0.0, OP.add, OP.max)

    lg_ps = f["gpsum"].tile([1, E], F32, tag="gps", name=n("lg_ps"))
    nc.tensor.matmul(lg_ps[:], lhsT=h_ext[:], rhs=wg2_sb, start=True, stop=True)

    u = g.tile([1, E], F32, tag="u", name=n("u"))
    nc.scalar.activation(u[:], lg_ps[:], ACTF.Exp)
    usum = g.tile([1, 1], F32, tag="usum", name=n("usum"))
    nc.vector.tensor_reduce(usum[:], u[:], axis=AX.X, op=OP.add)
    m1p = g.tile([1, 1], F32, tag="m1p", name=n("m1p"))
    nc.vector.tensor_reduce(m1p[:], u[:], axis=AX.X, op=OP.max)
    pm = g.tile([1, E], F32, tag="pm", name=n("pm"))
    nc.vector.scalar_tensor_tensor(pm[:], u[:], m1p[:], u[:], op0=OP.is_lt, op1=OP.mult)
    m2 = g.tile([1, 1], F32, tag="m2", name=n("m2"))
    nc.vector.tensor_reduce(m2[:], pm[:], axis=AX.X, op=OP.max)
    spv = g.tile([1, E], F32, tag="spv", name=n("spv"))
    nc.vector.scalar_tensor_tensor(spv[:], u[:], m2[:], u[:], op0=OP.is_ge, op1=OP.mult)
    dsum = g.tile([1, 1], F32, tag="dsum", name=n("dsum"))
    nc.vector.tensor_reduce(dsum[:], spv[:], axis=AX.X, op=OP.add)
    dd = g.tile([1, 1], F32, tag="dd", name=n("dd"))
    nc.vector.scalar_tensor_tensor(dd[:], usum[:], 1e-8, dsum[:], op0=OP.mult, op1=OP.add)
    rr = g.tile([1, 1], F32, tag="rr", name=n("rr"))
    nc.vector.reciprocal(rr[:], dd[:])
    w8 = g.tile([1, E], F32, tag="w8", name=n("w8"))
    nc.vector.tensor_scalar_mul(w8[:], spv[:], rr[:])

    # broadcast w8 down all 128 partitions, then stage to SBUF for MACs
    wb_ps = f["gpsum"].tile([128, E], F32, tag="gps", name=n("wb_ps"))
    nc.tensor.matmul(wb_ps[:], lhsT=ones[:], rhs=w8[:], start=True, stop=True)
    wb_sb = g.tile([128, E], F32, tag="wb_sb", name=n("wb_sb"))
    nc.vector.tensor_copy(wb_sb[:], wb_ps[:])

    # combined bias: b_comb = b_exp^T @ w8^T
    w8c_ps = f["gpsum"].tile([E, 1], F32, tag="gps", name=n("w8c_ps"))
    nc.tensor.matmul(w8c_ps[:], lhsT=w8[:], rhs=ones[:, 0:1], start=True, stop=True)
    w8col = g.tile([E, 1], F32, tag="w8col", name=n("w8col"))
    nc.vector.tensor_copy(w8col[:], w8c_ps[:])
    bc_ps = f["gpsum"].tile([C, 1], F32, tag="gps", name=n("bc_ps"))
    nc.tensor.matmul(bc_ps[:], lhsT=bexp_sb, rhs=w8col[:], start=True, stop=True)
    b_comb = g.tile([C, 1], F32, tag="b_comb", name=n("b_comb"))
    nc.vector.tensor_copy(b_comb[:], bc_ps[:])
    return wb_sb, b_comb


def _emit_mac(nc, pools, s, wb_sb, wpsA_sb, wpsB_sb):
    """wcomb = sum_e p_e wps_e: single DVE MAC chain accumulating in bf16
    (all-16-bit operands keep the DVE 2x mode; gpsimd lacks TSP support).
    Residual identity is pre-folded into every expert's center-tap B-half
    on the host."""
    f = pools
    wcombr = f["wcomb"].tile([128, 3, 128], BF16, tag="wcombr", name=f"wcombr{s}")
    nc.vector.tensor_scalar_mul(wcombr[:], wpsA_sb[:, 0], wb_sb[:, 0:1])
    for e in range(1, E):
        src = wpsA_sb[:, e] if e < 4 else wpsB_sb[:, e - 4]
        nc.vector.scalar_tensor_tensor(
            wcombr[:], src, wb_sb[:, e : e + 1], wcombr[:],
            op0=OP.mult, op1=OP.add,
        )
    return wcombr


def _emit_pair(nc, pools, s, p, XX, wcombr, b_comb, obuf, ocol):
    """Conv for tile pair p: 6 (or 3) matmuls into a 2-bank PSUM tile,
    dyi-major so consecutive matmuls share lhsT (LDWEIGHTS reuse). ACT
    stages the B half (+1 col) to SBUF bf16, DVE combines."""
    f = pools
    XX3 = XX[:, 0:FLAT].rearrange("p (r c) -> p r c", c=WP)
    tl = PAIRS[p]
    ps = f["cpsum"].tile([128, 2, 512], F32, tag="cps", name=f"cps{s}_{p}")
    for dyi in range(3):
        for t01, t in enumerate(tl):
            r0, nr = TILES[t]
            nc.tensor.matmul(
                ps[:, t01, 0 : nr * WP],
                lhsT=wcombr[:, dyi, :],
                rhs=XX3[:, r0 + dyi : r0 + dyi + nr, :],
                start=(dyi == 0),
                stop=(dyi == 2),
            )
    if len(tl) == 2:
        sbB = f["stage"].tile([128, 2, 390], BF16, tag="sbB", name=f"sbB{s}_{p}")
        nc.scalar.activation(sbB[0:64], ps[64:128, :, 1:391], ACTF.Copy)
        nc.vector.scalar_tensor_tensor(
            obuf[:, ocol : ocol + 780].rearrange("p (t c) -> p t c", c=390),
            ps[0:64, :, 0:390],
            b_comb[:],
            sbB[0:64],
            op0=OP.add,
            op1=OP.add,
        )
    else:
        ncols = TILES[tl[0]][1] * WP
        sbB = f["stage"].tile([128, 2, 390], BF16, tag="sbB", name=f"sbB{s}_{p}")
        nc.scalar.activation(
            sbB[0:64, 0, 0:ncols], ps[64:128, 0, 1 : 1 + ncols], ACTF.Copy
        )
        nc.vector.scalar_tensor_tensor(
            obuf[:, ocol : ocol + ncols],
            ps[0:64, 0, 0:ncols],
            b_comb[:],
            sbB[0:64, 0, 0:ncols],
            op0=OP.add,
            op1=OP.add,
        )


def build_program():
    if "nc" in _cache:
        return _cache["nc"]
    nc = bacc.Bacc("TRN2", target_bir_lowering=False, debug=False, enable_asserts=False)
    xs_ap = nc.dram_tensor("xs", [SPB, C, FLAT + 2], BF16, kind="ExternalInput").ap()
    wpsA_d = nc.dram_tensor("wpsA", [128, E // 2, 3, 128], BF16, kind="ExternalInput").ap()
    wpsB_d = nc.dram_tensor("wpsB", [128, E // 2, 3, 128], BF16, kind="ExternalInput").ap()
    gconst_d = nc.dram_tensor("gconst", [128, 90], F32, kind="ExternalInput").ap()
    out_ap = nc.dram_tensor("out", [SPB, C, H * WP], BF16, kind="ExternalOutput").ap()

    with tile.TileContext(nc) as tc, ExitStack() as ctx:
        pools = {
            "const": ctx.enter_context(tc.tile_pool(name="const", bufs=1)),
            "xx": ctx.enter_context(tc.tile_pool(name="xx", bufs=SPB)),
            "gate": ctx.enter_context(tc.tile_pool(name="gate", bufs=2)),
            "wcomb": ctx.enter_context(tc.tile_pool(name="wcomb", bufs=2)),
            "stage": ctx.enter_context(tc.tile_pool(name="stage", bufs=6)),
            "gpsum": ctx.enter_context(tc.tile_pool(name="gpsum", bufs=1, space="PSUM")),
            "cpsum": ctx.enter_context(tc.tile_pool(name="cpsum", bufs=3, space="PSUM")),
        }
        cp = pools["const"]
        XX0 = pools["xx"].tile([128, FLAT], BF16, tag="XX", name="XX0")
        XX1 = pools["xx"].tile([128, FLAT], BF16, tag="XX", name="XX1")
        gconst_sb = cp.tile([128, 90], F32)
        nc.gpsimd.dma_start(gconst_sb[:], gconst_d[:])
        ones = cp.tile([1, 128], F32)
        nc.gpsimd.memset(ones[:], 1.0)
        # prewarm the ACT exp table before the ACT lane fills with work
        warm = cp.tile([1, 1], F32)
        nc.scalar.activation(warm[:], ones[:, 0:1], ACTF.Exp)
        wpsA_sb = cp.tile([128, E // 2, 3, 128], BF16)
        wpsB_sb = cp.tile([128, E // 2, 3, 128], BF16)
        pools["scrD"] = cp.tile([128, QC + 2], BF16, name="scrD")
        pools["scrS"] = cp.tile([128, QC + 2], BF16, name="scrS")

        wg1x2_sb = gconst_sb[:, 0:16]
        bg1_sb = gconst_sb[0:16, 16:17]
        wg2_sb = gconst_sb[0:17, 17:25]
        bexp_sb = gconst_sb[0:8, 25:89]
        consts = (wg1x2_sb, bg1_sb, wg2_sb, bexp_sb, ones)

        # ---- loads ----
        # SP: s0 tops + wps; gpsimd: s1 tops; ACT: bottoms, GAP-feeding
        # chunks (2,3) of both samples before the conv-only chunks (0,1)
        for q in range(4):
            nc.sync.dma_start(
                XX0[0:64, QC * q : QC * (q + 1)], xs_ap[0, :, QC * q : QC * (q + 1)]
            )
        nc.sync.dma_start(wpsA_sb[:], wpsA_d[:])
        nc.sync.dma_start(wpsB_sb[:], wpsB_d[:])
        for q in range(4):
            nc.gpsimd.dma_start(
                XX1[0:64, QC * q : QC * (q + 1)], xs_ap[1, :, QC * q : QC * (q + 1)]
            )

        def bot_dma(s, XX, q):
            nc.scalar.dma_start(
                XX[64:128, QC * q : QC * (q + 1)],
                xs_ap[s, :, QC * q + 2 : QC * (q + 1) + 2],
            )

        bot_dma(0, XX0, 2)
        bot_dma(0, XX0, 3)
        # s0 top GAP on ACT (slipped before the remaining bottom issues)
        part0 = pools["gate"].tile([128, 2], F32, tag="part", name="part0")
        for win in GAP_TOP:
            _emit_gap_op(nc, pools, XX0, part0, win, is_bot=False, eng="act")
        bot_dma(1, XX1, 2)
        bot_dma(1, XX1, 3)
        bot_dma(0, XX0, 0)
        bot_dma(0, XX0, 1)
        bot_dma(1, XX1, 0)
        bot_dma(1, XX1, 1)
        # s0 bottom GAP on DVE
        for win in GAP_BOT:
            _emit_gap_op(nc, pools, XX0, part0, win, is_bot=True, eng="dve")
        pooled0 = pools["gate"].tile([128, 1], F32, tag="pooled", name="pooled0")
        nc.vector.tensor_reduce(pooled0, part0[:], axis=AX.X, op=OP.add)
        wb0, bcomb0 = _emit_gate(nc, pools, 0, pooled0, consts)
        wcombr0 = _emit_mac(nc, pools, 0, wb0, wpsA_sb, wpsB_sb)

        part1 = pools["gate"].tile([128, 2], F32, tag="part", name="part1")

        # s1 GAP on DVE, one op every few pairs (absorbed by the PE-paced
        # slack between combines)
        s1_gap_plan = {
            2: (GAP_TOP[0], False),
            5: (GAP_TOP[1], False),
            8: (GAP_BOT[0], True),
            11: (GAP_BOT[1], True),
        }

        def s1_gap_hook(p):
            if p in s1_gap_plan:
                win, is_bot = s1_gap_plan[p]
                _emit_gap_op(nc, pools, XX1, part1, win, is_bot=is_bot, eng="dve")

        obatch = {0: [None, 0, 0, 0], 1: [None, 0, 0, 0]}

        def emit_sample_pairs(s, XX, wcombr, bcomb, rng, hook=None):
            for p in rng:
                ob, orow, r0, nrows = obatch[s]
                if ob is None:
                    nrows = 24 if p + 4 <= 21 else (128 - 24 * 5)
                    r0 = TILES[PAIRS[p][0]][0]
                    ob = pools["stage"].tile(
                        [64, nrows * WP], BF16, tag="obuf", name=f"ob{s}_{p}",
                        bufs=3,
                    )
                    obatch[s] = [ob, 0, r0, nrows]
                    orow = 0
                _emit_pair(nc, pools, s, p, XX, wcombr, bcomb, ob, orow * WP)
                obatch[s][1] = orow = orow + sum(TILES[t][1] for t in PAIRS[p])
                if orow == nrows:
                    nc.gpsimd.dma_start(
                        out_ap[s, :, r0 * WP : (r0 + nrows) * WP], ob[:]
                    )
                    obatch[s] = [None, 0, 0, 0]
                if hook is not None:
                    hook(p)

        emit_sample_pairs(0, XX0, wcombr0, bcomb0, range(GATE_SPLIT), s1_gap_hook)
        pooled1 = pools["gate"].tile([128, 1], F32, tag="pooled", name="pooled1")
        nc.vector.tensor_reduce(pooled1, part1[:], axis=AX.X, op=OP.add)
        wb1, bcomb1 = _emit_gate(nc, pools, 1, pooled1, consts)
        wcombr1 = _emit_mac(nc, pools, 1, wb1, wpsA_sb, wpsB_sb)
        emit_sample_pairs(0, XX0, wcombr0, bcomb0, range(GATE_SPLIT, len(PAIRS)))
        emit_sample_pairs(1, XX1, wcombr1, bcomb1, range(len(PAIRS)))

    nc.compile()
    _cache["nc"] = nc
    return nc


def host_prep(x, wg1, bg1, wg2, bg2, w_exp, b_exp):
    """Host-side layout prep + per-core sharding. Returns in_maps list."""
    x = np.asarray(x, dtype=np.float32)
    wg1 = np.asarray(wg1, dtype=np.float32)
    bg1 = np.asarray(bg1, dtype=np.float32)
    wg2 = np.asarray(wg2, dtype=np.float32)
    bg2 = np.asarray(bg2, dtype=np.float32)
    w_exp = np.asarray(w_exp, dtype=np.float32)
    b_exp = np.asarray(b_exp, dtype=np.float32)

    # pre-padded x: [B, C, 130*130 + 2] bf16, SAME-conv zero border baked
    # in; +2 zero tail so the shifted bottom-half read stays in bounds
    xpad = np.zeros((B, C, HP, WP), np.float32)
    xpad[:, :, 1 : H + 1, 1 : W + 1] = x
    xs = np.zeros((B, C, FLAT + 2), NPBF16)
    xs[:, :, 0:FLAT] = xpad.reshape(B, C, FLAT).astype(NPBF16)

    # wps [128, E, 3(dy), 128]: K top/bottom = taps dx 0/2 on M 0:64 (A),
    # center dx=1 on M 64:128 top (B, bottom zero). Residual identity is
    # folded into every expert's center tap (sum of probs is ~1).
    wt = np.transpose(w_exp, (2, 0, 3, 4, 1))  # [I, E, dy, dx, O]
    wps = np.zeros((128, E, 3, 128), np.float32)
    wps[0:64, :, :, 0:64] = wt[:, :, :, 0, :]
    wps[64:128, :, :, 0:64] = wt[:, :, :, 2, :]
    wps[0:64, :, :, 64:128] = wt[:, :, :, 1, :]
    ii = np.arange(64)
    wps[ii, :, 1, 64 + ii] += 1.0

    gconst = np.zeros((128, 90), np.float32)
    gconst[:, 0:16] = np.concatenate([wg1, wg1], axis=0) / (H * W)
    gconst[0:16, 16] = bg1
    gconst[0:16, 17:25] = wg2
    gconst[16, 17:25] = bg2
    gconst[0:8, 25:89] = b_exp

    shared = {
        "wpsA": np.ascontiguousarray(wps[:, 0:4]).astype(NPBF16),
        "wpsB": np.ascontiguousarray(wps[:, 4:8]).astype(NPBF16),
        "gconst": gconst,
    }
    return [
        {"xs": np.ascontiguousarray(xs[SPB * k : SPB * (k + 1)]), **shared}
        for k in range(NCORES)
    ]


def kernel(x, wg1, bg1, wg2, bg2, w_exp, b_exp):
    nc = build_program()
    in_maps = host_prep(x, wg1, bg1, wg2, bg2, w_exp, b_exp)
    res = run_bass_kernel_spmd(nc, in_maps, list(range(NCORES)))
    outs = []
    for k in range(NCORES):
        o = np.asarray(res.results[k]["out"]).astype(np.float32)
        o = o.reshape(SPB, C, H, WP)[:, :, :, 0:W]
        outs.append(o)
    return np.concatenate(outs, axis=0)
